# revision 11
# baseline (speedup 1.0000x reference)
"""Cross-attention kernel for Trainium2, distributed over 8 NeuronCores.

Problem: B=4, Sk=4096, Sq=2048, d_model=1024, dims=64 (fp32 reference).

Sharding (hardcoded): core c -> (batch b = c//2, decoder half h = c%2).
Each core computes out[b, h*1024:(h+1)*1024, :] from enc[b] and its decoder
slice. No collectives.

Per-core dataflow (all layouts chosen so no large on-chip transposes are
needed):
  - Host pre-transposes/casts activations to bf16 and packs them so every
    big DMA reads contiguous 8KB-per-partition runs: encA [p, kp, h, c, k],
    decA [p, h, c, k] (d_model chunk c*128+p on partitions).
  - KV^T projection: lhsT = [Wv | Wk] [128d, 128], rhs = encT chunks
    -> psum [128, 512] where rows 0:64 = V^T, 64:128 = K^T. Full PE array.
  - V^T is evacuated into a tile whose row 64 is constant 1.0; PE transposes
    yield V-natural blocks [128k, 65] whose col 64 is the ones column ->
    AV lhsT directly, so the ones column accumulates the softmax denominator
    during the AV matmul (no separate row-sum pass).
  - Scores computed transposed: S^T[k,q] = (K Q^T) with K^T/q operands
    duplicated on both partition halves so the two matmuls of a k-block pair
    run CONCURRENTLY via PE row-tiling (tile_position (0,0) / (64,0)).
    exp(S^T) on ACT (PSUM->SBUF bf16) feeds the AV matmul as the moving
    operand. No max-subtraction (|scores| ~ N(0,1), exp is safe in fp32).
    The body is a software pipeline (kv chunk ck+1 | S/exp of ck | AV of
    ck-1) because Tile compiles a fixed per-engine execution order.
  - out^T [65, q] accumulated in PSUM over k blocks; the unnormalized
    [65, 1024] block (row 64 = softmax denominator) is stored and the
    normalization + final transpose happen on the host.
  - Startup: DMA descriptor enqueue is split across the sync (enc),
    scalar (dec) and gpsimd (constants) queues, big streams first, so the
    HBM stream starts ~2us in. Junk warmup matmuls (on a zeroed tile) are
    interleaved through the DMA-bound prologue to keep the PE HAM activity
    monitor busy so the PE clock un-throttles (1.2 -> 2.4 GHz) early.
"""

import numpy as np
import ml_dtypes
from einops import rearrange as _re

import concourse.bass as bass
import concourse.bacc as bacc
import concourse.tile as tile
from concourse import mybir
from concourse._compat import with_exitstack
from concourse.bass_utils import run_bass_kernel_spmd


BF16 = mybir.dt.bfloat16
F32 = mybir.dt.float32

B, SK, SQ_FULL, D, DIMS = 4, 4096, 2048, 1024, 64
N_CORES = 8
SQ = SQ_FULL * B // N_CORES  # 1024 decoder rows per core
DC = D // 128  # d_model chunks of 128
KPAIRS = SK // 1024  # 4 enc column pair-tiles
KBLKS = SK // 128  # 32 k blocks for attention
OBLKS = SQ // 128  # 8 output row blocks


@with_exitstack
def _body(ctx, tc, encA, decA, wkv, wq, bv, bk, bq, out):
    nc = tc.nc

    singles = ctx.enter_context(tc.tile_pool(name="singles", bufs=1))
    loads = ctx.enter_context(tc.tile_pool(name="loads", bufs=1))
    ps_pool = ctx.enter_context(tc.tile_pool(name="ps", bufs=2, space="PSUM"))
    po_pool = ctx.enter_context(tc.tile_pool(name="po", bufs=2, space="PSUM"))
    at_pool = ctx.enter_context(tc.tile_pool(name="at", bufs=6))
    outs = ctx.enter_context(tc.tile_pool(name="outs", bufs=1))

    # --- activation loads first: their descriptor enqueue gates everything.
    # sync enqueues enc (8MB), scalar enqueues dec (2MB); every transfer is
    # contiguous per partition in DRAM thanks to the host-side packing ---
    esbs = []
    for kp in range(KPAIRS):
        e0 = loads.tile([128, 4, 1024], BF16, tag=f"esb{kp}a", name=f"esb{kp}a")
        e1 = loads.tile([128, 4, 1024], BF16, tag=f"esb{kp}b", name=f"esb{kp}b")
        esbs.append((e0, e1))
    dsb = loads.tile([128, DC, SQ], BF16, tag="dload")

    # pair 0 split into d-quarters so the first kv matmuls start early
    for h in range(2):
        for q in range(2):
            nc.sync.dma_start(
                out=esbs[0][h][:, 2 * q:2 * q + 2, :],
                in_=encA[:, 0, h, 2 * q:2 * q + 2, :],
            )
    for h in range(2):
        nc.scalar.dma_start(
            out=dsb[:, h * 4:(h + 1) * 4, :], in_=decA[:, h, :, :]
        )
    for kp in range(1, KPAIRS):
        for h in range(2):
            nc.sync.dma_start(out=esbs[kp][h], in_=encA[:, kp, h, :, :])

    # --- constants (small, on the SWDGE/gpsimd queue so the big activation
    # streams own the HWDGE enqueue slots) ---
    wkv_sb = singles.tile([128, DC, 128], BF16)
    nc.gpsimd.dma_start(out=wkv_sb, in_=wkv)
    wq_sb = singles.tile([128, DC, 128], BF16)
    nc.gpsimd.dma_start(out=wq_sb, in_=wq)
    bv_sb = singles.tile([DIMS, 1], F32)
    nc.gpsimd.dma_start(out=bv_sb, in_=bv)
    bk_sb = singles.tile([DIMS, 1], F32)
    nc.gpsimd.dma_start(out=bk_sb, in_=bk)
    bq_sb = singles.tile([128, 1], F32)
    nc.gpsimd.dma_start(out=bq_sb, in_=bq)

    # --- PE warmup: junk matmuls on a zeroed tile keep the HAM activity
    # window busy during the DMA-bound prologue so the clock gate opens
    # (1.2 -> 2.4 GHz) before the real matmul stream begins ---
    junk = singles.tile([128, 256], BF16)
    nc.vector.memset(junk, 0.0)

    # --- persistent activations ---
    # K^T and the q operands are duplicated on both partition halves; the two
    # matmuls of an S pair run on disjoint PE row groups (concurrent).
    kTd = singles.tile([128, SK], BF16)
    # V^T with a ones row baked in at row 64.
    vTx = singles.tile([80, SK], BF16)
    nc.gpsimd.memset(vTx[64:80, :], 1.0)
    # V natural blocks: vnat[p, c, 0:64] = V[c*128+p, :], col 64 = 1.0
    vnat = singles.tile([128, KBLKS, 80], BF16)
    qTd = singles.tile([128, SQ], BF16)

    # full-height po tiles: AV accumulates in partitions 0:65; the junk
    # warmup matmuls write partitions 96:128 (their own PE column group,
    # disjoint per-element has_written bits -> no effect on the accumulation)
    po0 = po_pool.tile([128, 512], F32, tag="po")
    po1 = po_pool.tile([128, 512], F32, tag="po")
    pos = [po0, po1]

    def junk_mm(n):
        for _ in range(n):
            nc.tensor.matmul(
                po0[96:128, 0:256], lhsT=junk[:, 0:32], rhs=junk,
                start=True, stop=True, skip_group_check=True,
                tile_position=(0, 96),
            )

    # --- K/V projection + V transpose for one 512-column chunk (4 k blocks)
    def kv_chunk(ck):
        pskv = ps_pool.tile([128, 512], F32, tag="aux", name=f"pskv{ck % 2}")
        for d in range(DC):
            esb = esbs[ck // 2][d // 4]
            nc.tensor.matmul(
                pskv, lhsT=wkv_sb[:, d, :],
                rhs=esb[:, d % 4, (ck % 2) * 512:(ck % 2 + 1) * 512],
                start=(d == 0), stop=(d == DC - 1),
            )
        sl = slice(ck * 512, (ck + 1) * 512)
        nc.vector.tensor_scalar_add(vTx[0:DIMS, sl], pskv[0:DIMS, :], bv_sb)
        nc.vector.tensor_scalar_add(kTd[0:DIMS, sl], pskv[DIMS:128, :], bk_sb)
        nc.vector.tensor_scalar_add(kTd[DIMS:128, sl], pskv[DIMS:128, :], bk_sb)
        for kb in range(ck * 4, (ck + 1) * 4):
            nc.sync.dma_start(
                out=vnat[:, kb, :],
                in_=vTx[:, kb * 128:(kb + 1) * 128],
                transpose=True,
            )

    # --- attention for one pair of k blocks: 2 concurrent S matmuls
    # (row-tiled on array halves), 2 exps, 4 AV matmuls ---
    at_tiles = {}

    def s_exp_group(kg):
        psses = []
        for kb in (2 * kg, 2 * kg + 1):
            pss = ps_pool.tile([128, 2, 512], F32, tag="ps", name=f"pss{kb % 2}")
            psses.append(pss)
            for j in range(2):
                hp = DIMS * j
                nc.tensor.matmul(
                    pss[:, j, :], lhsT=kTd[hp:hp + DIMS, kb * 128:(kb + 1) * 128],
                    rhs=qTd[hp:hp + DIMS, j * 512:(j + 1) * 512],
                    start=True, stop=True,
                )
        for i, kb in enumerate((2 * kg, 2 * kg + 1)):
            at = at_pool.tile([128, 2, 512], BF16, tag="at", name=f"at{kb % 4}")
            at_tiles[kb] = at
            nc.scalar.activation(at, psses[i], mybir.ActivationFunctionType.Exp)

    def av_group(kg):
        for kb in (2 * kg, 2 * kg + 1):
            at = at_tiles.pop(kb)
            for j in range(2):
                nc.tensor.matmul(
                    pos[j][0:DIMS + 1, :], lhsT=vnat[:, kb, 0:DIMS + 1],
                    rhs=at[:, j, :],
                    start=(kb == 0), stop=(kb == KBLKS - 1),
                    skip_group_check=True,
                )

    # --- software pipeline: kv chunk ck+1 is produced while attention
    # consumes chunk ck; program order IS the per-engine execution order, so
    # the interleave keeps PE and ACT fed while DMA streams ---
    # prologue: kv chunk 0 and the Q projection interleaved by d-quarter to
    # match the DMA arrival order, with junk matmuls wedged in so the PE
    # stays HAM-busy while it waits for data
    pskv0 = ps_pool.tile([128, 512], F32, tag="aux", name="pskv0p")
    psq = ps_pool.tile([128, 2, 512], F32, tag="ps", name="psq")
    junk_mm(10)
    for half in range(2):
        for dq in range(2):
            for d in range(half * 4 + dq * 2, half * 4 + dq * 2 + 2):
                nc.tensor.matmul(
                    pskv0, lhsT=wkv_sb[:, d, :],
                    rhs=esbs[0][d // 4][:, d % 4, 0:512],
                    start=(d == 0), stop=(d == DC - 1),
                )
            junk_mm(3)
        for d in range(half * 4, half * 4 + 4):
            for j in range(2):
                nc.tensor.matmul(
                    psq[:, j, :], lhsT=wq_sb[:, d, :],
                    rhs=dsb[:, d, j * 512:(j + 1) * 512],
                    start=(d == 0), stop=(d == DC - 1),
                )
        junk_mm(4)
    nc.vector.tensor_scalar_add(vTx[0:DIMS, 0:512], pskv0[0:DIMS, :], bv_sb)
    nc.vector.tensor_scalar_add(kTd[0:DIMS, 0:512], pskv0[DIMS:128, :], bk_sb)
    nc.vector.tensor_scalar_add(kTd[DIMS:128, 0:512], pskv0[DIMS:128, :], bk_sb)
    for j in range(2):
        nc.vector.tensor_scalar_add(qTd[:, j * 512:(j + 1) * 512], psq[:, j, :], bq_sb)
    for kb in range(4):
        nc.sync.dma_start(
            out=vnat[:, kb, :],
            in_=vTx[:, kb * 128:(kb + 1) * 128],
            transpose=True,
        )

    NCK = SK // 512
    for ck in range(NCK):
        s_exp_group(2 * ck)
        if ck < 3:
            junk_mm(3)
        s_exp_group(2 * ck + 1)
        if ck + 1 < NCK:
            kv_chunk(ck + 1)
        if ck < 3:
            junk_mm(3)
        if ck > 0:
            av_group(2 * (ck - 1))
            av_group(2 * (ck - 1) + 1)
    av_group(2 * (NCK - 1))
    av_group(2 * (NCK - 1) + 1)

    # --- output: evacuate the unnormalized [65, 1024] block (row 64 holds
    # the softmax denominator) and store it; normalization and the final
    # transpose happen on the host ---
    out_sb = outs.tile([DIMS + 1, 2, 512], F32, tag="osb")
    for j in range(2):
        nc.vector.tensor_copy(out_sb[:, j, :], pos[j][0:DIMS + 1, :])
    nc.sync.dma_start(out=out, in_=out_sb)


_NC_CACHE = None


def _build():
    global _NC_CACHE
    if _NC_CACHE is not None:
        return _NC_CACHE
    nc = bacc.Bacc(
        "TRN2", target_bir_lowering=False, debug=False,
        enable_asserts=True, num_devices=N_CORES,
    )
    encA = nc.dram_tensor("encA", [128, KPAIRS, 2, 4, 1024], BF16,
                          kind="ExternalInput").ap()
    decA = nc.dram_tensor("decA", [128, 2, 4, SQ], BF16,
                          kind="ExternalInput").ap()
    wkv = nc.dram_tensor("wkv", [128, DC, 128], BF16, kind="ExternalInput").ap()
    wq = nc.dram_tensor("wq", [128, DC, 128], BF16, kind="ExternalInput").ap()
    bv = nc.dram_tensor("bv", [DIMS, 1], F32, kind="ExternalInput").ap()
    bk = nc.dram_tensor("bk", [DIMS, 1], F32, kind="ExternalInput").ap()
    bq = nc.dram_tensor("bq", [128, 1], F32, kind="ExternalInput").ap()
    out = nc.dram_tensor("out", [DIMS + 1, 2, 512], F32,
                         kind="ExternalOutput").ap()
    with tile.TileContext(nc) as tc:
        _body(tc, encA, decA, wkv, wq, bv, bk, bq, out)
    nc.compile()
    _NC_CACHE = nc
    return nc


def make_in_maps(**inputs):
    bf16 = ml_dtypes.bfloat16
    enc = np.asarray(inputs["encoder_output"])
    dec = np.asarray(inputs["decoder"])
    scale = DIMS ** -0.5
    wq1 = np.asarray(inputs["Wq"]) * scale
    wq_s = np.concatenate([wq1, wq1], axis=1).astype(bf16)
    wq_s = _re(wq_s, "(c p) m -> p c m", p=128)
    bq1 = (np.asarray(inputs["bq"]) * scale).astype(np.float32).reshape(DIMS, 1)
    bq_s = np.concatenate([bq1, bq1], axis=0)
    wkv = np.concatenate(
        [np.asarray(inputs["Wv"]), np.asarray(inputs["Wk"])], axis=1
    ).astype(bf16)
    wkv = _re(wkv, "(c p) m -> p c m", p=128)
    bv = np.asarray(inputs["bv"]).astype(np.float32).reshape(DIMS, 1)
    bk = np.asarray(inputs["bk"]).astype(np.float32).reshape(DIMS, 1)
    in_maps = []
    for c in range(N_CORES):
        b, h = divmod(c, 2)
        encA = _re(np.ascontiguousarray(enc[b].T).astype(bf16),
                   "(h c p) (kp k) -> p kp h c k", h=2, c=4, p=128, k=1024)
        decT = np.ascontiguousarray(dec[b, h * SQ:(h + 1) * SQ, :].T).astype(bf16)
        decA = _re(decT, "(h c p) k -> p h c k", h=2, c=4, p=128)
        in_maps.append({
            "encA": np.ascontiguousarray(encA),
            "decA": np.ascontiguousarray(decA),
            "wkv": wkv, "wq": wq_s, "bv": bv, "bk": bk, "bq": bq_s,
        })
    return in_maps


def assemble(results):
    out = np.zeros((B, SQ_FULL, DIMS), np.float32)
    for c in range(N_CORES):
        b, h = divmod(c, 2)
        o = results[c]["out"].reshape(DIMS + 1, SQ)  # [65, 1024]
        out[b, h * SQ:(h + 1) * SQ] = (o[0:DIMS] / o[DIMS:DIMS + 1]).T
    return out


def kernel(**inputs) -> np.ndarray:
    nc = _build()
    in_maps = make_in_maps(**inputs)
    res = run_bass_kernel_spmd(nc, in_maps, core_ids=list(range(N_CORES)))
    return assemble(res.results)


# revision 17
# speedup vs baseline: 1.0087x; 1.0087x over previous
"""Cross-attention kernel for Trainium2, distributed over 8 NeuronCores.

Problem: B=4, Sk=4096, Sq=2048, d_model=1024, dims=64 (fp32 reference).

Sharding (hardcoded): core c -> (batch b = c//2, decoder half h = c%2).
Each core computes out[b, h*1024:(h+1)*1024, :] from enc[b] and its decoder
slice. No collectives.

Per-core dataflow (all layouts chosen so no large on-chip transposes are
needed):
  - Host pre-transposes/casts activations to bf16 and packs them so every
    big DMA reads contiguous 8KB-per-partition runs: encA [p, kp, h, c, k],
    decA [p, h, c, k] (d_model chunk c*128+p on partitions).
  - KV^T projection: lhsT = [Wv | Wk] [128d, 128], rhs = encT chunks
    -> psum [128, 512] where rows 0:64 = V^T, 64:128 = K^T. Full PE array.
  - V^T is evacuated into a tile whose row 64 is constant 1.0; PE transposes
    yield V-natural blocks [128k, 65] whose col 64 is the ones column ->
    AV lhsT directly, so the ones column accumulates the softmax denominator
    during the AV matmul (no separate row-sum pass).
  - Scores computed transposed: S^T[k,q] = (K Q^T) with K^T/q operands
    duplicated on both partition halves so the two matmuls of a k-block pair
    run CONCURRENTLY via PE row-tiling (tile_position (0,0) / (64,0)).
    exp(S^T) on ACT (PSUM->SBUF bf16) feeds the AV matmul as the moving
    operand. No max-subtraction (|scores| ~ N(0,1), exp is safe in fp32).
    The body is a software pipeline (kv chunk ck+1 | S/exp of ck | AV of
    ck-1) because Tile compiles a fixed per-engine execution order.
  - out^T [65, q] accumulated in PSUM over k blocks; the unnormalized
    [65, 1024] block (row 64 = softmax denominator) is stored and the
    normalization + final transpose happen on the host.
  - Startup: DMA descriptor enqueue is split across the sync (enc),
    scalar (dec) and gpsimd (constants) queues, big streams first, so the
    HBM stream starts ~2us in. Junk warmup matmuls (on a zeroed tile) are
    interleaved through the DMA-bound prologue to keep the PE HAM activity
    monitor busy so the PE clock un-throttles (1.2 -> 2.4 GHz) early.
"""

import numpy as np
import ml_dtypes
from einops import rearrange as _re

import concourse.bass as bass
import concourse.bacc as bacc
import concourse.tile as tile
from concourse import mybir
from concourse._compat import with_exitstack
from concourse.bass_utils import run_bass_kernel_spmd
from concourse.masks import make_identity

BF16 = mybir.dt.bfloat16
F32 = mybir.dt.float32

B, SK, SQ_FULL, D, DIMS = 4, 4096, 2048, 1024, 64
N_CORES = 8
SQ = SQ_FULL * B // N_CORES  # 1024 decoder rows per core
DC = D // 128  # d_model chunks of 128
KPAIRS = SK // 1024  # 4 enc column pair-tiles
KBLKS = SK // 128  # 32 k blocks for attention
OBLKS = SQ // 128  # 8 output row blocks


@with_exitstack
def _body(ctx, tc, encA, decA, wkv, wq, bv, bk, bq, out):
    nc = tc.nc

    singles = ctx.enter_context(tc.tile_pool(name="singles", bufs=1))
    loads = ctx.enter_context(tc.tile_pool(name="loads", bufs=1))
    ps_pool = ctx.enter_context(tc.tile_pool(name="ps", bufs=2, space="PSUM"))
    po_pool = ctx.enter_context(tc.tile_pool(name="po", bufs=2, space="PSUM"))
    at_pool = ctx.enter_context(tc.tile_pool(name="at", bufs=6))
    outs = ctx.enter_context(tc.tile_pool(name="outs", bufs=1))

    # --- activation loads first: their descriptor enqueue gates everything.
    # sync enqueues enc (8MB), scalar enqueues dec (2MB); every transfer is
    # contiguous per partition in DRAM thanks to the host-side packing ---
    esbs = []
    for kp in range(KPAIRS):
        e0 = loads.tile([128, 4, 1024], BF16, tag=f"esb{kp}a", name=f"esb{kp}a")
        e1 = loads.tile([128, 4, 1024], BF16, tag=f"esb{kp}b", name=f"esb{kp}b")
        esbs.append((e0, e1))
    dsb = loads.tile([128, DC, SQ], BF16, tag="dload")

    # pair 0 split into d-quarters so the first kv matmuls start early
    for h in range(2):
        for q in range(2):
            nc.sync.dma_start(
                out=esbs[0][h][:, 2 * q:2 * q + 2, :],
                in_=encA[:, 0, h, 2 * q:2 * q + 2, :],
            )
    for h in range(2):
        nc.scalar.dma_start(
            out=dsb[:, h * 4:(h + 1) * 4, :], in_=decA[:, h, :, :]
        )
    for kp in range(1, KPAIRS):
        for h in range(2):
            nc.sync.dma_start(out=esbs[kp][h], in_=encA[:, kp, h, :, :])

    # --- constants (small, on the SWDGE/gpsimd queue so the big activation
    # streams own the HWDGE enqueue slots) ---
    wkv_sb = singles.tile([128, DC, 128], BF16)
    nc.gpsimd.dma_start(out=wkv_sb, in_=wkv)
    wq_sb = singles.tile([128, DC, 128], BF16)
    nc.gpsimd.dma_start(out=wq_sb, in_=wq)
    bv_sb = singles.tile([DIMS, 1], F32)
    nc.gpsimd.dma_start(out=bv_sb, in_=bv)
    bk_sb = singles.tile([DIMS, 1], F32)
    nc.gpsimd.dma_start(out=bk_sb, in_=bk)
    bq_sb = singles.tile([128, 1], F32)
    nc.gpsimd.dma_start(out=bq_sb, in_=bq)
    ident_bf = singles.tile([128, 128], BF16)
    make_identity(nc, ident_bf)

    # --- PE warmup: junk matmuls on a zeroed tile keep the HAM activity
    # window busy during the DMA-bound prologue so the clock gate opens
    # (1.2 -> 2.4 GHz) before the real matmul stream begins ---
    junk = singles.tile([128, 256], BF16)
    nc.vector.memset(junk, 0.0)

    # --- persistent activations ---
    # K^T and the q operands are duplicated on both partition halves; the two
    # matmuls of an S pair run on disjoint PE row groups (concurrent).
    kTd = singles.tile([128, SK], BF16)
    # V^T with a ones row baked in at row 64.
    vTx = singles.tile([80, SK], BF16)
    nc.gpsimd.memset(vTx[64:80, :], 1.0)
    # V natural blocks: vnat[p, c, 0:64] = V[c*128+p, :], col 64 = 1.0
    vnat = singles.tile([128, KBLKS, 80], BF16)
    qTd = singles.tile([128, SQ], BF16)

    # full-height po tiles: AV accumulates in partitions 0:65; the junk
    # warmup matmuls write partitions 96:128 (their own PE column group,
    # disjoint per-element has_written bits -> no effect on the accumulation)
    po0 = po_pool.tile([128, 512], F32, tag="po")
    po1 = po_pool.tile([128, 512], F32, tag="po")
    pos = [po0, po1]

    def junk_mm(n):
        for _ in range(n):
            nc.tensor.matmul(
                po0[96:128, 0:256], lhsT=junk[:, 0:32], rhs=junk,
                start=True, stop=True, skip_group_check=True,
                tile_position=(0, 96),
            )

    # --- K/V projection + V transpose for one 512-column chunk (4 k blocks)
    def kv_chunk(ck):
        pskv = ps_pool.tile([128, 512], F32, tag="aux", name=f"pskv{ck % 2}")
        for d in range(DC):
            esb = esbs[ck // 2][d // 4]
            nc.tensor.matmul(
                pskv, lhsT=wkv_sb[:, d, :],
                rhs=esb[:, d % 4, (ck % 2) * 512:(ck % 2 + 1) * 512],
                start=(d == 0), stop=(d == DC - 1),
            )
        sl = slice(ck * 512, (ck + 1) * 512)
        nc.vector.tensor_scalar_add(vTx[0:DIMS, sl], pskv[0:DIMS, :], bv_sb)
        nc.vector.tensor_scalar_add(kTd[0:DIMS, sl], pskv[DIMS:128, :], bk_sb)
        nc.vector.tensor_scalar_add(kTd[DIMS:128, sl], pskv[DIMS:128, :], bk_sb)
        for kb in range(ck * 4, (ck + 1) * 4):
            ptv = ps_pool.tile([128, DIMS + 1], BF16, tag="aux", name=f"ptv{kb % 2}")
            nc.tensor.transpose(
                ptv, vTx[0:DIMS + 1, kb * 128:(kb + 1) * 128],
                ident_bf[0:DIMS + 1, 0:DIMS + 1],
            )
            nc.vector.tensor_copy(vnat[:, kb, 0:DIMS + 1], ptv)

    # --- attention for one pair of k blocks: 2 concurrent S matmuls
    # (row-tiled on array halves), 2 exps, 4 AV matmuls ---
    at_tiles = {}

    def s_exp_group(kg):
        psses = []
        for kb in (2 * kg, 2 * kg + 1):
            pss = ps_pool.tile([128, 2, 512], F32, tag="ps", name=f"pss{kb % 2}")
            psses.append(pss)
            for j in range(2):
                hp = DIMS * j
                nc.tensor.matmul(
                    pss[:, j, :], lhsT=kTd[hp:hp + DIMS, kb * 128:(kb + 1) * 128],
                    rhs=qTd[hp:hp + DIMS, j * 512:(j + 1) * 512],
                    start=True, stop=True,
                )
        for i, kb in enumerate((2 * kg, 2 * kg + 1)):
            at = at_pool.tile([128, 2, 512], BF16, tag="at", name=f"at{kb % 4}")
            at_tiles[kb] = at
            nc.scalar.activation(at, psses[i], mybir.ActivationFunctionType.Exp)

    def av_group(kg):
        for kb in (2 * kg, 2 * kg + 1):
            at = at_tiles.pop(kb)
            for j in range(2):
                nc.tensor.matmul(
                    pos[j][0:DIMS + 1, :], lhsT=vnat[:, kb, 0:DIMS + 1],
                    rhs=at[:, j, :],
                    start=(kb == 0), stop=(kb == KBLKS - 1),
                    skip_group_check=True,
                )

    # --- software pipeline: kv chunk ck+1 is produced while attention
    # consumes chunk ck; program order IS the per-engine execution order, so
    # the interleave keeps PE and ACT fed while DMA streams ---
    # prologue: kv chunk 0 and the Q projection interleaved by d-quarter to
    # match the DMA arrival order, with junk matmuls wedged in so the PE
    # stays HAM-busy while it waits for data
    pskv0 = ps_pool.tile([128, 512], F32, tag="aux", name="pskv0p")
    psq = ps_pool.tile([128, 2, 512], F32, tag="ps", name="psq")
    # contiguous warmup block: ~3.5us of sustained PE busy flips the HAM
    # clock gate to 8/8 before the first data-dependent matmul; the HBM
    # stream hasn't delivered anything yet, so this costs nothing
    junk_mm(26)
    for half in range(2):
        for dq in range(2):
            for d in range(half * 4 + dq * 2, half * 4 + dq * 2 + 2):
                nc.tensor.matmul(
                    pskv0, lhsT=wkv_sb[:, d, :],
                    rhs=esbs[0][d // 4][:, d % 4, 0:512],
                    start=(d == 0), stop=(d == DC - 1),
                )
            junk_mm(4)
        for d in range(half * 4, half * 4 + 4):
            for j in range(2):
                nc.tensor.matmul(
                    psq[:, j, :], lhsT=wq_sb[:, d, :],
                    rhs=dsb[:, d, j * 512:(j + 1) * 512],
                    start=(d == 0), stop=(d == DC - 1),
                )
        junk_mm(4)
    nc.vector.tensor_scalar_add(vTx[0:DIMS, 0:512], pskv0[0:DIMS, :], bv_sb)
    nc.vector.tensor_scalar_add(kTd[0:DIMS, 0:512], pskv0[DIMS:128, :], bk_sb)
    nc.vector.tensor_scalar_add(kTd[DIMS:128, 0:512], pskv0[DIMS:128, :], bk_sb)
    for j in range(2):
        nc.vector.tensor_scalar_add(qTd[:, j * 512:(j + 1) * 512], psq[:, j, :], bq_sb)
    for kb in range(4):
        ptv = ps_pool.tile([128, DIMS + 1], BF16, tag="aux", name=f"ptv{kb % 2}")
        nc.tensor.transpose(
            ptv, vTx[0:DIMS + 1, kb * 128:(kb + 1) * 128],
            ident_bf[0:DIMS + 1, 0:DIMS + 1],
        )
        nc.vector.tensor_copy(vnat[:, kb, 0:DIMS + 1], ptv)

    NCK = SK // 512
    for ck in range(NCK):
        s_exp_group(2 * ck)
        s_exp_group(2 * ck + 1)
        if ck in (1, 2):
            # early iterations are HBM-stream-bound: run the ready AV work
            # (and junk filler) BEFORE stalling on the next enc chunk
            av_group(2 * (ck - 1))
            av_group(2 * (ck - 1) + 1)
            junk_mm(10)
            kv_chunk(ck + 1)
        else:
            if ck + 1 < NCK:
                kv_chunk(ck + 1)
            if ck > 0:
                av_group(2 * (ck - 1))
                av_group(2 * (ck - 1) + 1)
    av_group(2 * (NCK - 1))
    av_group(2 * (NCK - 1) + 1)

    # --- output: evacuate the unnormalized [65, 1024] block (row 64 holds
    # the softmax denominator) and store it; normalization and the final
    # transpose happen on the host ---
    out_sb = outs.tile([DIMS + 1, 2, 512], F32, tag="osb")
    for j in range(2):
        nc.vector.tensor_copy(out_sb[:, j, :], pos[j][0:DIMS + 1, :])
    nc.sync.dma_start(out=out, in_=out_sb)


_NC_CACHE = None


def _build():
    global _NC_CACHE
    if _NC_CACHE is not None:
        return _NC_CACHE
    nc = bacc.Bacc(
        "TRN2", target_bir_lowering=False, debug=False,
        enable_asserts=True, num_devices=N_CORES,
    )
    encA = nc.dram_tensor("encA", [128, KPAIRS, 2, 4, 1024], BF16,
                          kind="ExternalInput").ap()
    decA = nc.dram_tensor("decA", [128, 2, 4, SQ], BF16,
                          kind="ExternalInput").ap()
    wkv = nc.dram_tensor("wkv", [128, DC, 128], BF16, kind="ExternalInput").ap()
    wq = nc.dram_tensor("wq", [128, DC, 128], BF16, kind="ExternalInput").ap()
    bv = nc.dram_tensor("bv", [DIMS, 1], F32, kind="ExternalInput").ap()
    bk = nc.dram_tensor("bk", [DIMS, 1], F32, kind="ExternalInput").ap()
    bq = nc.dram_tensor("bq", [128, 1], F32, kind="ExternalInput").ap()
    out = nc.dram_tensor("out", [DIMS + 1, 2, 512], F32,
                         kind="ExternalOutput").ap()
    with tile.TileContext(nc) as tc:
        _body(tc, encA, decA, wkv, wq, bv, bk, bq, out)
    nc.compile()
    _NC_CACHE = nc
    return nc


def make_in_maps(**inputs):
    bf16 = ml_dtypes.bfloat16
    enc = np.asarray(inputs["encoder_output"])
    dec = np.asarray(inputs["decoder"])
    scale = DIMS ** -0.5
    wq1 = np.asarray(inputs["Wq"]) * scale
    wq_s = np.concatenate([wq1, wq1], axis=1).astype(bf16)
    wq_s = _re(wq_s, "(c p) m -> p c m", p=128)
    bq1 = (np.asarray(inputs["bq"]) * scale).astype(np.float32).reshape(DIMS, 1)
    bq_s = np.concatenate([bq1, bq1], axis=0)
    wkv = np.concatenate(
        [np.asarray(inputs["Wv"]), np.asarray(inputs["Wk"])], axis=1
    ).astype(bf16)
    wkv = _re(wkv, "(c p) m -> p c m", p=128)
    bv = np.asarray(inputs["bv"]).astype(np.float32).reshape(DIMS, 1)
    bk = np.asarray(inputs["bk"]).astype(np.float32).reshape(DIMS, 1)
    in_maps = []
    for c in range(N_CORES):
        b, h = divmod(c, 2)
        encA = _re(np.ascontiguousarray(enc[b].T).astype(bf16),
                   "(h c p) (kp k) -> p kp h c k", h=2, c=4, p=128, k=1024)
        decT = np.ascontiguousarray(dec[b, h * SQ:(h + 1) * SQ, :].T).astype(bf16)
        decA = _re(decT, "(h c p) k -> p h c k", h=2, c=4, p=128)
        in_maps.append({
            "encA": np.ascontiguousarray(encA),
            "decA": np.ascontiguousarray(decA),
            "wkv": wkv, "wq": wq_s, "bv": bv, "bk": bk, "bq": bq_s,
        })
    return in_maps


def assemble(results):
    out = np.zeros((B, SQ_FULL, DIMS), np.float32)
    for c in range(N_CORES):
        b, h = divmod(c, 2)
        o = results[c]["out"].reshape(DIMS + 1, SQ)  # [65, 1024]
        out[b, h * SQ:(h + 1) * SQ] = (o[0:DIMS] / o[DIMS:DIMS + 1]).T
    return out


def kernel(**inputs) -> np.ndarray:
    nc = _build()
    in_maps = make_in_maps(**inputs)
    res = run_bass_kernel_spmd(nc, in_maps, core_ids=list(range(N_CORES)))
    return assemble(res.results)
